# revision 15
# baseline (speedup 1.0000x reference)
"""Gated causal attention (B=2, L=2048, HID=2048, NH=16, HD=128) on 8 trn2 cores.

Sharding: data-parallel over batch (cores 0-3 batch 0, cores 4-7 batch 1) x
tensor-parallel over heads (4 heads per core within its batch). Each core:
  - projects q/k/v/g for its 4 heads (fp32r matmuls, x.T resident in SBUF)
  - RoPE on q/k in [d, m] layout (rotate-half via SBUF->SBUF swap DMA)
  - causal attention per head in S_T = [kpos, q] layout; softmax denominators
    via an all-ones stationary matmul; no max-subtraction (scores are small)
  - per-head RMSNorm + silu gating on broadcast [128, m] tiles
  - o_proj partial [L, 2048]

Runner: the axon tunnel (~70 MB/s up, ~50 MB/s down, ~85 ms RTT, partially
full-duplex) dominates wall time, so all static tensors (weights, RoPE tables,
masks) are uploaded once and cached device-resident. Per call only
hidden_states moves host->device (fp16), an on-device prep jit (all_gather +
transpose) builds each core's x^T, the bass custom call runs, and an on-device
psum_scatter reduces the o_proj partials across the 4 head-groups before an
fp16 fetch of the final output. Three separate jits because the bass_exec
compile hook requires its jit to be exactly params -> custom-call. The two
batches run as independent 4-core submesh programs so batch 1's upload
overlaps batch 0's downlink, and fetches are per-shard on a thread pool.
"""

import numpy as np

B, L, HID, NH, HD = 2, 2048, 2048, 16, 128
EPS = 1e-5
SCALE = HD ** -0.5
ROPE_BASE = 10000.0
NCORES = 8
HPC = 4            # heads per core
NDIM = HPC * HD    # 512 projection dims per core
P = 128
KC = HID // P      # 16 k-chunks
CC = L // P        # 16 kpos chunks
QT = 512           # q tile (fp32r moving max)
NHALF = L // 2     # AV/den psum half width
NCH = (4 * NDIM) // P  # 16 fused projection n-chunks (q|k|v|g)


def _build(nc, mybir, tile):
    from contextlib import ExitStack

    f32 = mybir.dt.float32
    f32r = mybir.dt.float32r
    AF = mybir.ActivationFunctionType
    OP = mybir.AluOpType

    xT = nc.dram_tensor("xT", [HID, L], f32r, kind="ExternalInput")
    # wT blocked: [k-chunk, n-chunk, 128, 128]; n order = q|k|v|g, each 512
    wTb = nc.dram_tensor("wTb", [KC, NCH, P, P], f32r, kind="ExternalInput")
    woT = nc.dram_tensor("woT", [NDIM, HID], f32r, kind="ExternalInput")
    cosq = nc.dram_tensor("cosq", [P, L], f32, kind="ExternalInput")
    ssinq = nc.dram_tensor("ssinq", [P, L], f32, kind="ExternalInput")
    cosk = nc.dram_tensor("cosk", [P, L], f32, kind="ExternalInput")
    ssink = nc.dram_tensor("ssink", [P, L], f32, kind="ExternalInput")
    ones_t = nc.dram_tensor("ones_t", [P, P], f32r, kind="ExternalInput")
    oneshd_t = nc.dram_tensor("oneshd_t", [P, P], f32r, kind="ExternalInput")
    ident_t = nc.dram_tensor("ident_t", [P, P], f32r, kind="ExternalInput")
    masks_t = nc.dram_tensor("masks_t", [4, P, QT], f32r, kind="ExternalInput")
    nw_t = nc.dram_tensor("nw_t", [P, 1], f32, kind="ExternalInput")
    out_partial = nc.dram_tensor("out_partial", [L, HID], f32,
                                 kind="ExternalOutput")

    with tile.TileContext(nc) as tc, ExitStack() as octx:
        const = octx.enter_context(tc.tile_pool(name="const", bufs=1))
        ones = const.tile([P, P], f32r, tag="ones")
        oneshd = const.tile([P, P], f32r, tag="oneshd")
        ident = const.tile([P, P], f32r, tag="ident")
        nw = const.tile([P, 1], f32, tag="nw")
        masks = [const.tile([P, QT], f32r, tag=f"mask{r}", name=f"mask{r}") for r in range(4)]

        # DRAM staging pools (tracked by Tile)
        dstage = octx.enter_context(tc.tile_pool(name="stage", bufs=1,
                                                 space="DRAM"))
        qkvg = [dstage.tile([P, L], f32r, tag=f"qkvg{n}", name=f"qkvg{n}") for n in range(NCH)]
        gstage = [dstage.tile([P, L], f32r, tag=f"gst{h}", name=f"gst{h}") for h in range(HPC)]

        # ================= Phase A: projections =================
        with ExitStack() as ctx:
            xpool = ctx.enter_context(tc.tile_pool(name="xt", bufs=1))
            xt = [None] * KC

            wpool = ctx.enter_context(tc.tile_pool(name="wc", bufs=4))
            ppool = ctx.enter_context(
                tc.tile_pool(name="proj_psum", bufs=2, space="PSUM"))
            epool = ctx.enter_context(tc.tile_pool(name="evict", bufs=2))
            tabpool = ctx.enter_context(tc.tile_pool(name="tables", bufs=1))

            cos_tab = sin_tab = None
            for n in range(NCH):
                if n == 0 or n == 4:
                    cos_tab = tabpool.tile([P, L], f32, tag="cos")
                    sin_tab = tabpool.tile([P, L], f32, tag="sin")
                    nc.sync.dma_start(cos_tab[:], cosq[:] if n == 0 else cosk[:])
                    nc.sync.dma_start(sin_tab[:], ssinq[:] if n == 0 else ssink[:])
                psum = ppool.tile([P, L], f32, tag="pp")
                for k in range(KC):
                    if xt[k] is None:
                        t = xpool.tile([P, L], f32r, tag=f"xt{k}",
                                       name=f"xtile{k}")
                        nc.sync.dma_start(t[:], xT[k * P:(k + 1) * P, :])
                        xt[k] = t
                    wc = wpool.tile([P, P], f32r, tag="wc")
                    nc.sync.dma_start(wc[:], wTb[k, n])
                    for mt in range(L // QT):
                        nc.tensor.matmul(
                            psum[:, mt * QT:(mt + 1) * QT],
                            wc[:],
                            xt[k][:, mt * QT:(mt + 1) * QT],
                            start=(k == 0),
                            stop=(k == KC - 1),
                        )
                for hf in range(2):
                    sl = slice(hf * NHALF, (hf + 1) * NHALF)
                    if n < 8:
                        raw = epool.tile([P, NHALF], f32, tag="raw")
                        nc.vector.tensor_copy(raw[:], psum[:, sl])
                        swp = epool.tile([P, NHALF], f32, tag="swp")
                        nc.sync.dma_start(swp[:64, :], raw[64:, :])
                        nc.sync.dma_start(swp[64:, :], raw[:64, :])
                        nc.vector.tensor_mul(raw[:], raw[:], cos_tab[:, sl])
                        nc.vector.tensor_mul(swp[:], swp[:], sin_tab[:, sl])
                        roped = epool.tile([P, NHALF], f32r, tag="roped")
                        nc.vector.tensor_add(roped[:], raw[:], swp[:])
                        nc.sync.dma_start(qkvg[n][:, sl], roped[:])
                    else:
                        ev = epool.tile([P, NHALF], f32r, tag="roped")
                        nc.scalar.copy(ev[:], psum[:, sl])
                        nc.sync.dma_start(qkvg[n][:, sl], ev[:])

        nc.sync.dma_start(ones[:], ones_t[:])
        nc.sync.dma_start(oneshd[:], oneshd_t[:])
        nc.sync.dma_start(ident[:], ident_t[:])
        nc.sync.dma_start(nw[:], nw_t[:])
        for r in range(4):
            nc.sync.dma_start(masks[r][:], masks_t[r])

        # ================= Phase B: attention per head =================
        with ExitStack() as ctx:
            hpool2 = ctx.enter_context(tc.tile_pool(name="headio2", bufs=2))
            hpool1 = ctx.enter_context(tc.tile_pool(name="headio1", bufs=1))
            vtp = ctx.enter_context(
                tc.tile_pool(name="vt_psum", bufs=1, space="PSUM"))
            vnpool = ctx.enter_context(tc.tile_pool(name="vnat", bufs=1))
            stp = ctx.enter_context(
                tc.tile_pool(name="st_psum", bufs=2, space="PSUM"))
            ptpool = ctx.enter_context(tc.tile_pool(name="pt", bufs=1))
            avp = ctx.enter_context(
                tc.tile_pool(name="av_psum", bufs=1, space="PSUM"))
            denp = ctx.enter_context(
                tc.tile_pool(name="den_psum", bufs=1, space="PSUM"))
            epi = ctx.enter_context(tc.tile_pool(name="epi", bufs=1))

            for h in range(HPC):
                qTt = hpool2.tile([P, L], f32r, tag="qT")
                kTt = hpool2.tile([P, L], f32r, tag="kT")
                vTt = hpool1.tile([P, L], f32r, tag="vT")
                nc.sync.dma_start(qTt[:], qkvg[h][:])
                nc.sync.dma_start(kTt[:], qkvg[4 + h][:])
                nc.sync.dma_start(vTt[:], qkvg[8 + h][:])

                vnat = []
                for c in range(CC):
                    vt_ps = vtp.tile([P, P], f32r, tag="vtp")
                    nc.tensor.transpose(
                        vt_ps[:], vTt[:, c * P:(c + 1) * P], ident[:])
                    vn = vnpool.tile([P, P], f32r, tag=f"vn{c}")
                    nc.vector.tensor_copy(vn[:], vt_ps[:])
                    vnat.append(vn)

                gTt = hpool1.tile([P, L], f32r, tag="gT")
                nc.sync.dma_start(gTt[:], qkvg[12 + h][:])
                gt = hpool1.tile([P, L], f32r, tag="gated")

                # S_T + exp + mask + AV, interleaved per kpos chunk
                av = avp.tile([P, L], f32, tag="av")
                pts = []
                for c in range(CC):
                    qs = QT * (c // 4)
                    pt = ptpool.tile([P, L - qs], f32r, tag=f"pt{c}")
                    for j in range(c // 4, L // QT):
                        ps = stp.tile([P, QT], f32, tag="st")
                        nc.tensor.matmul(
                            ps[:],
                            kTt[:, c * P:(c + 1) * P],
                            qTt[:, j * QT:(j + 1) * QT],
                            start=True, stop=True,
                        )
                        nc.scalar.activation(
                            pt[:, j * QT - qs:(j + 1) * QT - qs], ps[:], AF.Exp)
                    nc.vector.tensor_mul(
                        pt[:, 0:QT], pt[:, 0:QT], masks[c % 4][:])
                    pts.append(pt)
                    for j in range(c // 4, L // QT):
                        nc.tensor.matmul(
                            av[:, j * QT:(j + 1) * QT],
                            vnat[c][:],
                            pt[:, j * QT - qs:(j + 1) * QT - qs],
                            start=(c == 0),
                            stop=(c == 4 * j + 3),
                        )

                # evictions (DVE) + silu (ACT)
                rawh = epi.tile([P, L], f32, tag="rawh")
                nc.vector.tensor_copy(rawh[:], av[:])
                sqh = epi.tile([P, L], f32r, tag="sqh")
                nc.vector.tensor_mul(sqh[:], rawh[:], rawh[:])
                sgh = epi.tile([P, L], f32, tag="sgh")
                nc.scalar.activation(sgh[:], gTt[:], AF.Silu)
                cbh = epi.tile([P, L], f32, tag="cbh")

                # den + rms, 512-wide quarters; batch same-ACT-func ops
                dens, d2s, t2s = [], [], []
                for qq in range(L // QT):
                    den = denp.tile([P, QT], f32, tag="den")
                    for c in range(4 * qq + 4):
                        qs = QT * (c // 4)
                        nc.tensor.matmul(
                            den[:],
                            ones[:],
                            pts[c][:, qq * QT - qs:(qq + 1) * QT - qs],
                            start=(c == 0),
                            stop=(c == 4 * qq + 3),
                        )
                    dens.append(den)
                for qq in range(L // QT):
                    d2 = epi.tile([P, QT], f32, tag=f"d2_{qq}")
                    nc.scalar.activation(d2[:], dens[qq][:], AF.Square)
                    d2s.append(d2)
                for qq in range(L // QT):
                    sl = slice(qq * QT, (qq + 1) * QT)
                    s2 = stp.tile([P, QT], f32, tag="st")
                    nc.tensor.matmul(s2[:], oneshd[:], sqh[:, sl],
                                     start=True, stop=True)
                    t2 = epi.tile([P, QT], f32, tag=f"t2_{qq}")
                    nc.vector.scalar_tensor_tensor(
                        t2[:], d2s[qq][:], float(EPS), s2[:],
                        op0=OP.mult, op1=OP.add)
                    t2s.append(t2)
                for qq in range(L // QT):
                    nc.scalar.activation(t2s[qq][:], t2s[qq][:], AF.Sqrt)
                for qq in range(L // QT):
                    sl = slice(qq * QT, (qq + 1) * QT)
                    nc.vector.reciprocal(cbh[:, sl], t2s[qq][:])

                nc.vector.tensor_mul(rawh[:], rawh[:], cbh[:])
                nc.vector.scalar_tensor_tensor(
                    gt[:], rawh[:], nw[:], sgh[:],
                    op0=OP.mult, op1=OP.mult)
                nc.sync.dma_start(gstage[h][:], gt[:])

        # ================= Phase C: o_proj =================
        with ExitStack() as ctx:
            wop = ctx.enter_context(tc.tile_pool(name="wo", bufs=1))
            gpool = ctx.enter_context(tc.tile_pool(name="gres", bufs=1))
            wot, gres = [], []
            for h in range(HPC):
                t = wop.tile([P, HID], f32r, tag=f"wo{h}")
                nc.sync.dma_start(t[:], woT[h * P:(h + 1) * P, :])
                wot.append(t)
                g = gpool.tile([P, L], f32r, tag=f"gr{h}")
                nc.sync.dma_start(g[:], gstage[h][:])
                gres.append(g)
            opp = ctx.enter_context(
                tc.tile_pool(name="oproj_psum", bufs=2, space="PSUM"))
            oev = ctx.enter_context(tc.tile_pool(name="oev", bufs=3))
            for mc in range(L // P):
                ops = opp.tile([P, HID], f32, tag="op")
                for h in range(HPC):
                    for s in range(HID // QT):
                        nc.tensor.matmul(
                            ops[:, s * QT:(s + 1) * QT],
                            gres[h][:, mc * P:(mc + 1) * P],
                            wot[h][:, s * QT:(s + 1) * QT],
                            start=(h == 0),
                            stop=(h == HPC - 1),
                        )
                ot = oev.tile([P, HID], f32, tag="ot")
                nc.scalar.copy(ot[:], ops[:])
                nc.sync.dma_start(out_partial[mc * P:(mc + 1) * P, :], ot[:])

    return nc


def _rope_tables():
    inv_freq = 1.0 / (ROPE_BASE ** (np.arange(0, HD, 2, dtype=np.float64) / HD))
    t = np.arange(L, dtype=np.float64)
    f = np.outer(inv_freq, t)                      # [64, L]
    cosT = np.concatenate([np.cos(f), np.cos(f)], 0)
    ssinT = np.concatenate([-np.sin(f), np.sin(f)], 0)
    cosq = np.ascontiguousarray((cosT * SCALE).astype(np.float32))
    ssinq = np.ascontiguousarray((ssinT * SCALE).astype(np.float32))
    cosk = np.ascontiguousarray(cosT.astype(np.float32))
    ssink = np.ascontiguousarray(ssinT.astype(np.float32))
    return cosq, ssinq, cosk, ssink


def _static_in_maps(wq, wk, wv, wg, wo, norm_w):
    """Per-core input maps for everything except xT (static across calls)."""
    cosq, ssinq, cosk, ssink = _rope_tables()
    ones = np.ones((P, P), np.float32)
    oneshd = np.full((P, P), 1.0 / HD, np.float32)
    ident = np.eye(P, dtype=np.float32)
    qq = np.arange(QT)[None, :]
    kk = np.arange(P)[:, None]
    masks = np.ascontiguousarray(
        np.stack([(qq >= P * r + kk) for r in range(4)]).astype(np.float32))
    nw = np.ascontiguousarray(norm_w.astype(np.float32).reshape(P, 1))

    per_hg = []
    for hg in range(4):
        hs = slice(NDIM * hg, NDIM * (hg + 1))
        W = np.concatenate([wq[hs], wk[hs], wv[hs], wg[hs]], 0)
        wT = np.ascontiguousarray(np.asarray(W).T.astype(np.float32))
        wTb = np.ascontiguousarray(
            wT.reshape(KC, P, NCH, P).transpose(0, 2, 1, 3))
        woTc = np.ascontiguousarray(np.asarray(wo)[:, hs].T.astype(np.float32))
        per_hg.append((wTb, woTc))

    in_maps = []
    for c in range(NCORES):
        wTb, woTc = per_hg[c % 4]
        in_maps.append({
            "wTb": wTb, "woT": woTc,
            "cosq": cosq, "ssinq": ssinq, "cosk": cosk, "ssink": ssink,
            "ones_t": ones, "oneshd_t": oneshd, "ident_t": ident,
            "masks_t": masks, "nw_t": nw,
        })
    return in_maps


_ST = {}


def _get_runner(groups=(0, 1)):
    if "runner" in _ST:
        return _ST["runner"]

    import jax
    import jax.numpy as jnp
    from jax.sharding import Mesh, PartitionSpec as PS, NamedSharding
    try:
        from jax import shard_map as _sm

        def shard_map(f, **kw):
            return _sm(f, check_vma=False, **kw)
    except ImportError:
        from jax.experimental.shard_map import shard_map as _sm

        def shard_map(f, **kw):
            return _sm(f, check_rep=False, **kw)
    import concourse.bacc as bacc
    import concourse.mybir as mybir
    import concourse.tile as tile
    from concourse.bass2jax import (
        _bass_exec_p, install_neuronx_cc_hook, partition_id_tensor)

    install_neuronx_cc_hook()

    nc = bacc.Bacc("TRN2", target_bir_lowering=False, debug=False)
    _build(nc, mybir, tile)
    nc.compile()

    devs = jax.devices()[:NCORES]
    assert len(devs) == NCORES
    meshes = [Mesh(np.asarray(devs[4 * b:4 * b + 4]), ("h",))
              for b in range(B)]

    # Enumerate NEFF-bound tensors in BIR allocation order, exactly as
    # run_bass_via_pjrt does.
    partition_name = (nc.partition_id_tensor.name
                      if nc.partition_id_tensor else None)
    in_names, out_names, out_avals = [], [], []
    for alloc in nc.m.functions[0].allocations:
        if not isinstance(alloc, mybir.MemoryLocationSet):
            continue
        name = alloc.memorylocations[0].name
        if alloc.kind == "ExternalInput":
            if name != partition_name:
                in_names.append(name)
        elif alloc.kind == "ExternalOutput":
            out_avals.append(jax.core.ShapedArray(
                tuple(alloc.tensor_shape), mybir.dt.np(alloc.dtype)))
            out_names.append(name)
    n_params = len(in_names)
    in_shapes = {}
    for alloc in nc.m.functions[0].allocations:
        if isinstance(alloc, mybir.MemoryLocationSet) and alloc.kind == "ExternalInput":
            in_shapes[alloc.memorylocations[0].name] = tuple(alloc.tensor_shape)
    bind_in_names = list(in_names)
    if partition_name is not None:
        bind_in_names.append(partition_name)

    def _body(*args):
        operands = list(args)
        if partition_name is not None:
            operands.append(partition_id_tensor())
        outs = _bass_exec_p.bind(
            *operands,
            out_avals=tuple(out_avals),
            in_names=tuple(bind_in_names),
            out_names=tuple(out_names),
            lowering_input_output_aliases=(),
            sim_require_finite=True,
            sim_require_nnan=True,
            nc=nc,
        )
        return tuple(outs)

    def _spec(rank):
        return PS("h", *([None] * (rank - 1)))

    bass_in_specs = tuple(_spec(len(in_shapes[n])) for n in in_names)
    bass_out_specs = tuple(_spec(len(a.shape)) for a in out_avals)

    # prep: per-device [L/4, HID] fp16 slice of the batch -> full x[b]^T f32
    def _prep_body(xl):
        xg = jax.lax.all_gather(xl, "h", axis=0, tiled=True)   # [L, HID]
        return xg.astype(jnp.float32).T                         # [HID, L]

    # reduce: sum o_proj partials over the 4 head-group cores, scatter rows
    def _red_body(y):
        z = jax.lax.psum_scatter(y, "h", scatter_dimension=0, tiled=True)
        return z.astype(jnp.float16)

    bass_jits, prep_jits, red_jits, x_shs, g_shs = (
        [None] * B, [None] * B, [None] * B, [None] * B, [None] * B)
    for b in groups:
        mesh = meshes[b]
        bass_jits[b] = jax.jit(shard_map(
            _body, mesh=mesh, in_specs=bass_in_specs,
            out_specs=bass_out_specs))
        prep_jits[b] = jax.jit(shard_map(
            _prep_body, mesh=mesh, in_specs=(PS("h", None),),
            out_specs=PS("h", None)))
        red_jits[b] = jax.jit(shard_map(
            _red_body, mesh=mesh, in_specs=(PS("h", None),),
            out_specs=PS("h", None)))
        x_shs[b] = NamedSharding(mesh, PS("h", None))
        g_shs[b] = lambda rank, m=mesh: NamedSharding(m, _spec(rank))

    from concurrent.futures import ThreadPoolExecutor
    runner = {
        "jax": jax, "meshes": meshes, "x_sh": x_shs, "groups": tuple(groups),
        "in_names": in_names, "global_sharding": g_shs,
        "prep": prep_jits, "bass": bass_jits, "red": red_jits,
        "pool": ThreadPoolExecutor(max_workers=2 * NCORES),
    }
    _ST["runner"] = runner
    return runner


def _get_statics(runner, wq, wk, wv, wg, wo, norm_w):
    key = (id(wq), id(wk), id(wv), id(wg), id(wo), id(norm_w))
    cached = _ST.get("statics")
    if cached is not None and cached[0] == key:
        return cached[1]
    jax = runner["jax"]
    in_maps = _static_in_maps(np.asarray(wq), np.asarray(wk), np.asarray(wv),
                              np.asarray(wg), np.asarray(wo),
                              np.asarray(norm_w))
    statics = [None] * B
    for b in runner["groups"]:
        sb = {}
        for name in runner["in_names"]:
            if name == "xT":
                continue
            g = np.concatenate(
                [in_maps[c][name] for c in range(4 * b, 4 * b + 4)], axis=0)
            sb[name] = jax.device_put(g, runner["global_sharding"][b](g.ndim))
        statics[b] = sb
    jax.block_until_ready(
        [v for sb in statics if sb for v in sb.values()])
    _ST["statics"] = (key, statics)
    return statics


def _fetch_shard_into(sdata, dst_rows):
    dst_rows[...] = np.asarray(sdata).astype(np.float32)


def _run_batch_group(runner, statics, b, x_b, out_b, pool):
    """Upload x_b (fp16 [L, HID]), run prep/bass/red for group b, fetch
    the fp16 result into out_b (fp32 [L, HID]). Returns fetch futures."""
    jax = runner["jax"]
    xd = jax.device_put(x_b, runner["x_sh"][b])
    xT = runner["prep"][b](xd)
    args = [xT if n == "xT" else statics[b][n] for n in runner["in_names"]]
    (partials,) = runner["bass"][b](*args)
    o16 = runner["red"][b](partials)              # [L, HID] fp16, 4 shards
    o16.copy_to_host_async()
    return [pool.submit(_fetch_shard_into, s.data, out_b[s.index[0]])
            for s in o16.addressable_shards]


def _kernel_fast(hidden_states, wq, wk, wv, wg, wo, norm_w):
    runner = _get_runner()
    statics = _get_statics(runner, wq, wk, wv, wg, wo, norm_w)
    pool = runner["pool"]
    x = np.asarray(hidden_states)
    out = np.empty((B, L, HID), np.float32)
    futs = []
    for b in range(B):
        futs += _run_batch_group(runner, statics, b, x[b].astype(np.float16),
                                 out[b], pool)
    for f in futs:
        f.result()
    return out


# ---------------- two-process runner ----------------
#
# The axon tunnel serializes transfers per client connection (~50-70 MB/s
# each direction), but two OS processes get independent connections with
# ~1.35x aggregate bandwidth. Each worker owns one batch on its 4 cores.
# Raw subprocess + shared memory (no multiprocessing: spawn would re-import
# an unguarded caller __main__, fork would inherit a live PJRT client).

_W_SHAPES = [(HID, HID)] * 5 + [(HD,)]
_W_WBYTES = sum(int(np.prod(s)) * 4 for s in _W_SHAPES)


def _worker_entry():
    import sys
    from multiprocessing import shared_memory
    from concurrent.futures import ThreadPoolExecutor

    b = int(sys.argv[1])
    kw = dict(track=False)
    try:
        shm_x = shared_memory.SharedMemory(name=sys.argv[2], **kw)
    except TypeError:
        kw = {}
        shm_x = shared_memory.SharedMemory(name=sys.argv[2])
    shm_o = shared_memory.SharedMemory(name=sys.argv[3], **kw)
    shm_w = shared_memory.SharedMemory(name=sys.argv[4], **kw)
    xbuf = np.ndarray((L, HID), np.float16, buffer=shm_x.buf)
    obuf = np.ndarray((L, HID), np.float32, buffer=shm_o.buf)
    pool = ThreadPoolExecutor(max_workers=8)
    runner = statics = None
    for line in sys.stdin:
        cmd = line.strip()
        try:
            if cmd == "init":
                runner = _get_runner(groups=(b,))
                ws, off = [], 0
                for shape in _W_SHAPES:
                    n = int(np.prod(shape))
                    ws.append(np.frombuffer(shm_w.buf, np.float32, n, off)
                              .reshape(shape).copy())
                    off += n * 4
                statics = _get_statics(runner, *ws)
                futs = _run_batch_group(
                    runner, statics, b, np.zeros((L, HID), np.float16),
                    obuf, pool)
                for f in futs:
                    f.result()
                print("ready", flush=True)
            elif cmd == "run":
                futs = _run_batch_group(runner, statics, b, xbuf, obuf, pool)
                for f in futs:
                    f.result()
                print("done", flush=True)
            elif cmd == "quit":
                break
        except Exception:
            import traceback
            traceback.print_exc(file=sys.stderr)
            print("error", flush=True)


def _stop_workers():
    w = _ST.pop("workers", None)
    if not w:
        return
    for p in w["procs"]:
        try:
            p.stdin.close()
            p.kill()
        except Exception:
            pass
    for shm in w["shms"]:
        try:
            shm.close()
            shm.unlink()
        except Exception:
            pass


def _await_token(w, token, timeout):
    for b in range(B):
        q = w["queues"][b]
        import queue as _qmod
        deadline = None
        while True:
            try:
                line = q.get(timeout=timeout)
            except _qmod.Empty:
                raise RuntimeError(f"worker {b}: timeout waiting {token}")
            if line is None:
                raise RuntimeError(f"worker {b}: exited")
            if line == token:
                break
            if line == "error":
                raise RuntimeError(f"worker {b}: reported error")


def _get_workers(wq, wk, wv, wg, wo, norm_w):
    key = tuple(id(a) for a in (wq, wk, wv, wg, wo, norm_w))
    w = _ST.get("workers")
    if w is not None:
        if w["key"] == key and all(p.poll() is None for p in w["procs"]):
            return w
        _stop_workers()
    import os
    import subprocess
    import sys
    import threading
    import queue
    from multiprocessing import shared_memory

    kdir = os.path.dirname(os.path.abspath(__file__))
    shm_w = shared_memory.SharedMemory(create=True, size=_W_WBYTES)
    off = 0
    for a in (wq, wk, wv, wg, wo, norm_w):
        a32 = np.ascontiguousarray(np.asarray(a), dtype=np.float32)
        np.frombuffer(shm_w.buf, np.float32, a32.size, off)[...] = a32.ravel()
        off += a32.nbytes
    procs, xbufs, obufs, queues, shms = [], [], [], [], [shm_w]
    for b in range(B):
        shm_x = shared_memory.SharedMemory(create=True, size=L * HID * 2)
        shm_o = shared_memory.SharedMemory(create=True, size=L * HID * 4)
        shms += [shm_x, shm_o]
        xbufs.append(np.ndarray((L, HID), np.float16, buffer=shm_x.buf))
        obufs.append(np.ndarray((L, HID), np.float32, buffer=shm_o.buf))
        p = subprocess.Popen(
            [sys.executable, "-c", "import kernel; kernel._worker_entry()",
             str(b), shm_x.name, shm_o.name, shm_w.name],
            cwd=kdir, stdin=subprocess.PIPE, stdout=subprocess.PIPE,
            text=True, bufsize=1)
        q = queue.Queue()

        def _reader(pipe=p.stdout, q=q):
            for line in pipe:
                s = line.strip()
                if s in ("ready", "done", "error"):
                    q.put(s)
            q.put(None)

        threading.Thread(target=_reader, daemon=True).start()
        procs.append(p)
        queues.append(q)
    w = {"key": key, "procs": procs, "xbufs": xbufs, "obufs": obufs,
         "queues": queues, "shms": shms}
    _ST["workers"] = w
    try:
        for p in procs:
            p.stdin.write("init\n")
            p.stdin.flush()
        _await_token(w, "ready", timeout=1800)
    except Exception:
        _stop_workers()
        raise
    return w


def _kernel_fast2(hidden_states, wq, wk, wv, wg, wo, norm_w):
    w = _get_workers(wq, wk, wv, wg, wo, norm_w)
    x = np.asarray(hidden_states)
    try:
        for b in range(B):
            np.copyto(w["xbufs"][b], x[b], casting="same_kind")
            w["procs"][b].stdin.write("run\n")
            w["procs"][b].stdin.flush()
        _await_token(w, "done", timeout=120)
    except Exception:
        _stop_workers()
        raise
    out = np.empty((B, L, HID), np.float32)
    for b in range(B):
        out[b] = w["obufs"][b]
    return out


# ---------------- fallback path (original runner) ----------------

def _host_inputs_full(hidden_states, wq, wk, wv, wg, wo, norm_w):
    x = np.ascontiguousarray(hidden_states.astype(np.float32))
    in_maps = _static_in_maps(wq, wk, wv, wg, wo, norm_w)
    for c in range(NCORES):
        in_maps[c] = dict(in_maps[c])
        in_maps[c]["xT"] = np.ascontiguousarray(x[c // 4].T)
    return in_maps


def _kernel_fallback(hidden_states, wq, wk, wv, wg, wo, norm_w):
    from concourse.bass_utils import run_bass_kernel_spmd
    import concourse.bacc as bacc
    import concourse.mybir as mybir
    import concourse.tile as tile

    if "nc_fb" not in _ST:
        nc = bacc.Bacc("TRN2", target_bir_lowering=False, debug=False)
        _build(nc, mybir, tile)
        nc.compile()
        _ST["nc_fb"] = nc
    nc = _ST["nc_fb"]
    in_maps = _host_inputs_full(np.asarray(hidden_states), np.asarray(wq),
                                np.asarray(wk), np.asarray(wv),
                                np.asarray(wg), np.asarray(wo),
                                np.asarray(norm_w))
    res = run_bass_kernel_spmd(nc, in_maps, list(range(NCORES)))
    out = np.zeros((B, L, HID), np.float32)
    for c in range(NCORES):
        out[c // 4] += res.results[c]["out_partial"]
    return out


def kernel(hidden_states, wq, wk, wv, wg, wo, norm_w, _trace=False):
    if not _ST.get("skip_fast2"):
        try:
            return _kernel_fast2(hidden_states, wq, wk, wv, wg, wo, norm_w)
        except Exception:
            import traceback
            traceback.print_exc()
            _ST["skip_fast2"] = True
    if not _ST.get("use_fallback"):
        try:
            return _kernel_fast(hidden_states, wq, wk, wv, wg, wo, norm_w)
        except Exception:
            import traceback
            traceback.print_exc()
            _ST["use_fallback"] = True
    return _kernel_fallback(hidden_states, wq, wk, wv, wg, wo, norm_w)


# revision 16
# speedup vs baseline: 1.1279x; 1.1279x over previous
"""Gated causal attention (B=2, L=2048, HID=2048, NH=16, HD=128) on 8 trn2 cores.

Sharding: data-parallel over batch (cores 0-3 batch 0, cores 4-7 batch 1) x
tensor-parallel over heads (4 heads per core within its batch). Each core:
  - projects q/k/v/g for its 4 heads (fp32r matmuls, x.T resident in SBUF)
  - RoPE on q/k in [d, m] layout (rotate-half via SBUF->SBUF swap DMA)
  - causal attention per head in S_T = [kpos, q] layout; softmax denominators
    via an all-ones stationary matmul; no max-subtraction (scores are small)
  - per-head RMSNorm + silu gating on broadcast [128, m] tiles
  - o_proj partial [L, 2048]

Runner: the axon tunnel (~70 MB/s up, ~50 MB/s down, ~85 ms RTT, full-duplex
across directions but single-stream within each) dominates wall time, so all
static tensors (weights, RoPE tables, masks) are uploaded once and cached
device-resident. Per call only hidden_states moves host->device (fp16), an
on-device prep jit (all_gather + transpose) builds each core's x^T, the bass
custom call runs, and an on-device psum_scatter reduces the o_proj partials
across the 4 head-groups before an fp16 fetch of the final output. Three
separate jits because the bass_exec compile hook requires its jit to be
exactly params -> custom-call. The two batches run as independent 4-core
submesh programs so batch 1's upload overlaps batch 0's downlink
(copy_to_host_async prefetches results), and fetches are per-shard on a
thread pool.
"""

import numpy as np

B, L, HID, NH, HD = 2, 2048, 2048, 16, 128
EPS = 1e-5
SCALE = HD ** -0.5
ROPE_BASE = 10000.0
NCORES = 8
HPC = 4            # heads per core
NDIM = HPC * HD    # 512 projection dims per core
P = 128
KC = HID // P      # 16 k-chunks
CC = L // P        # 16 kpos chunks
QT = 512           # q tile (fp32r moving max)
NHALF = L // 2     # AV/den psum half width
NCH = (4 * NDIM) // P  # 16 fused projection n-chunks (q|k|v|g)


def _build(nc, mybir, tile):
    from contextlib import ExitStack

    f32 = mybir.dt.float32
    f32r = mybir.dt.float32r
    AF = mybir.ActivationFunctionType
    OP = mybir.AluOpType

    xT = nc.dram_tensor("xT", [HID, L], f32r, kind="ExternalInput")
    # wT blocked: [k-chunk, n-chunk, 128, 128]; n order = q|k|v|g, each 512
    wTb = nc.dram_tensor("wTb", [KC, NCH, P, P], f32r, kind="ExternalInput")
    woT = nc.dram_tensor("woT", [NDIM, HID], f32r, kind="ExternalInput")
    cosq = nc.dram_tensor("cosq", [P, L], f32, kind="ExternalInput")
    ssinq = nc.dram_tensor("ssinq", [P, L], f32, kind="ExternalInput")
    cosk = nc.dram_tensor("cosk", [P, L], f32, kind="ExternalInput")
    ssink = nc.dram_tensor("ssink", [P, L], f32, kind="ExternalInput")
    ones_t = nc.dram_tensor("ones_t", [P, P], f32r, kind="ExternalInput")
    oneshd_t = nc.dram_tensor("oneshd_t", [P, P], f32r, kind="ExternalInput")
    ident_t = nc.dram_tensor("ident_t", [P, P], f32r, kind="ExternalInput")
    masks_t = nc.dram_tensor("masks_t", [4, P, QT], f32r, kind="ExternalInput")
    nw_t = nc.dram_tensor("nw_t", [P, 1], f32, kind="ExternalInput")
    out_partial = nc.dram_tensor("out_partial", [L, HID], f32,
                                 kind="ExternalOutput")

    with tile.TileContext(nc) as tc, ExitStack() as octx:
        const = octx.enter_context(tc.tile_pool(name="const", bufs=1))
        ones = const.tile([P, P], f32r, tag="ones")
        oneshd = const.tile([P, P], f32r, tag="oneshd")
        ident = const.tile([P, P], f32r, tag="ident")
        nw = const.tile([P, 1], f32, tag="nw")
        masks = [const.tile([P, QT], f32r, tag=f"mask{r}", name=f"mask{r}") for r in range(4)]

        # DRAM staging pools (tracked by Tile)
        dstage = octx.enter_context(tc.tile_pool(name="stage", bufs=1,
                                                 space="DRAM"))
        qkvg = [dstage.tile([P, L], f32r, tag=f"qkvg{n}", name=f"qkvg{n}") for n in range(NCH)]
        gstage = [dstage.tile([P, L], f32r, tag=f"gst{h}", name=f"gst{h}") for h in range(HPC)]

        # ================= Phase A: projections =================
        with ExitStack() as ctx:
            xpool = ctx.enter_context(tc.tile_pool(name="xt", bufs=1))
            xt = [None] * KC

            wpool = ctx.enter_context(tc.tile_pool(name="wc", bufs=4))
            ppool = ctx.enter_context(
                tc.tile_pool(name="proj_psum", bufs=2, space="PSUM"))
            epool = ctx.enter_context(tc.tile_pool(name="evict", bufs=2))
            tabpool = ctx.enter_context(tc.tile_pool(name="tables", bufs=1))

            cos_tab = sin_tab = None
            for n in range(NCH):
                if n == 0 or n == 4:
                    cos_tab = tabpool.tile([P, L], f32, tag="cos")
                    sin_tab = tabpool.tile([P, L], f32, tag="sin")
                    nc.sync.dma_start(cos_tab[:], cosq[:] if n == 0 else cosk[:])
                    nc.sync.dma_start(sin_tab[:], ssinq[:] if n == 0 else ssink[:])
                psum = ppool.tile([P, L], f32, tag="pp")
                for k in range(KC):
                    if xt[k] is None:
                        t = xpool.tile([P, L], f32r, tag=f"xt{k}",
                                       name=f"xtile{k}")
                        nc.sync.dma_start(t[:], xT[k * P:(k + 1) * P, :])
                        xt[k] = t
                    wc = wpool.tile([P, P], f32r, tag="wc")
                    nc.sync.dma_start(wc[:], wTb[k, n])
                    for mt in range(L // QT):
                        nc.tensor.matmul(
                            psum[:, mt * QT:(mt + 1) * QT],
                            wc[:],
                            xt[k][:, mt * QT:(mt + 1) * QT],
                            start=(k == 0),
                            stop=(k == KC - 1),
                        )
                for hf in range(2):
                    sl = slice(hf * NHALF, (hf + 1) * NHALF)
                    if n < 8:
                        raw = epool.tile([P, NHALF], f32, tag="raw")
                        nc.vector.tensor_copy(raw[:], psum[:, sl])
                        swp = epool.tile([P, NHALF], f32, tag="swp")
                        nc.sync.dma_start(swp[:64, :], raw[64:, :])
                        nc.sync.dma_start(swp[64:, :], raw[:64, :])
                        nc.vector.tensor_mul(raw[:], raw[:], cos_tab[:, sl])
                        nc.vector.tensor_mul(swp[:], swp[:], sin_tab[:, sl])
                        roped = epool.tile([P, NHALF], f32r, tag="roped")
                        nc.vector.tensor_add(roped[:], raw[:], swp[:])
                        nc.sync.dma_start(qkvg[n][:, sl], roped[:])
                    else:
                        ev = epool.tile([P, NHALF], f32r, tag="roped")
                        nc.scalar.copy(ev[:], psum[:, sl])
                        nc.sync.dma_start(qkvg[n][:, sl], ev[:])

        nc.sync.dma_start(ones[:], ones_t[:])
        nc.sync.dma_start(oneshd[:], oneshd_t[:])
        nc.sync.dma_start(ident[:], ident_t[:])
        nc.sync.dma_start(nw[:], nw_t[:])
        for r in range(4):
            nc.sync.dma_start(masks[r][:], masks_t[r])

        # ================= Phase B: attention per head =================
        with ExitStack() as ctx:
            hpool2 = ctx.enter_context(tc.tile_pool(name="headio2", bufs=2))
            hpool1 = ctx.enter_context(tc.tile_pool(name="headio1", bufs=1))
            vtp = ctx.enter_context(
                tc.tile_pool(name="vt_psum", bufs=1, space="PSUM"))
            vnpool = ctx.enter_context(tc.tile_pool(name="vnat", bufs=1))
            stp = ctx.enter_context(
                tc.tile_pool(name="st_psum", bufs=2, space="PSUM"))
            ptpool = ctx.enter_context(tc.tile_pool(name="pt", bufs=1))
            avp = ctx.enter_context(
                tc.tile_pool(name="av_psum", bufs=1, space="PSUM"))
            denp = ctx.enter_context(
                tc.tile_pool(name="den_psum", bufs=1, space="PSUM"))
            epi = ctx.enter_context(tc.tile_pool(name="epi", bufs=1))

            for h in range(HPC):
                qTt = hpool2.tile([P, L], f32r, tag="qT")
                kTt = hpool2.tile([P, L], f32r, tag="kT")
                vTt = hpool1.tile([P, L], f32r, tag="vT")
                nc.sync.dma_start(qTt[:], qkvg[h][:])
                nc.sync.dma_start(kTt[:], qkvg[4 + h][:])
                nc.sync.dma_start(vTt[:], qkvg[8 + h][:])

                vnat = []
                for c in range(CC):
                    vt_ps = vtp.tile([P, P], f32r, tag="vtp")
                    nc.tensor.transpose(
                        vt_ps[:], vTt[:, c * P:(c + 1) * P], ident[:])
                    vn = vnpool.tile([P, P], f32r, tag=f"vn{c}")
                    nc.vector.tensor_copy(vn[:], vt_ps[:])
                    vnat.append(vn)

                gTt = hpool1.tile([P, L], f32r, tag="gT")
                nc.sync.dma_start(gTt[:], qkvg[12 + h][:])
                gt = hpool1.tile([P, L], f32r, tag="gated")

                # S_T + exp + mask + AV, interleaved per kpos chunk
                av = avp.tile([P, L], f32, tag="av")
                pts = []
                for c in range(CC):
                    qs = QT * (c // 4)
                    pt = ptpool.tile([P, L - qs], f32r, tag=f"pt{c}")
                    for j in range(c // 4, L // QT):
                        ps = stp.tile([P, QT], f32, tag="st")
                        nc.tensor.matmul(
                            ps[:],
                            kTt[:, c * P:(c + 1) * P],
                            qTt[:, j * QT:(j + 1) * QT],
                            start=True, stop=True,
                        )
                        nc.scalar.activation(
                            pt[:, j * QT - qs:(j + 1) * QT - qs], ps[:], AF.Exp)
                    nc.vector.tensor_mul(
                        pt[:, 0:QT], pt[:, 0:QT], masks[c % 4][:])
                    pts.append(pt)
                    for j in range(c // 4, L // QT):
                        nc.tensor.matmul(
                            av[:, j * QT:(j + 1) * QT],
                            vnat[c][:],
                            pt[:, j * QT - qs:(j + 1) * QT - qs],
                            start=(c == 0),
                            stop=(c == 4 * j + 3),
                        )

                # evictions (DVE) + silu (ACT)
                rawh = epi.tile([P, L], f32, tag="rawh")
                nc.vector.tensor_copy(rawh[:], av[:])
                sqh = epi.tile([P, L], f32r, tag="sqh")
                nc.vector.tensor_mul(sqh[:], rawh[:], rawh[:])
                sgh = epi.tile([P, L], f32, tag="sgh")
                nc.scalar.activation(sgh[:], gTt[:], AF.Silu)
                cbh = epi.tile([P, L], f32, tag="cbh")

                # den + rms, 512-wide quarters; batch same-ACT-func ops
                dens, d2s, t2s = [], [], []
                for qq in range(L // QT):
                    den = denp.tile([P, QT], f32, tag="den")
                    for c in range(4 * qq + 4):
                        qs = QT * (c // 4)
                        nc.tensor.matmul(
                            den[:],
                            ones[:],
                            pts[c][:, qq * QT - qs:(qq + 1) * QT - qs],
                            start=(c == 0),
                            stop=(c == 4 * qq + 3),
                        )
                    dens.append(den)
                for qq in range(L // QT):
                    d2 = epi.tile([P, QT], f32, tag=f"d2_{qq}")
                    nc.scalar.activation(d2[:], dens[qq][:], AF.Square)
                    d2s.append(d2)
                for qq in range(L // QT):
                    sl = slice(qq * QT, (qq + 1) * QT)
                    s2 = stp.tile([P, QT], f32, tag="st")
                    nc.tensor.matmul(s2[:], oneshd[:], sqh[:, sl],
                                     start=True, stop=True)
                    t2 = epi.tile([P, QT], f32, tag=f"t2_{qq}")
                    nc.vector.scalar_tensor_tensor(
                        t2[:], d2s[qq][:], float(EPS), s2[:],
                        op0=OP.mult, op1=OP.add)
                    t2s.append(t2)
                for qq in range(L // QT):
                    nc.scalar.activation(t2s[qq][:], t2s[qq][:], AF.Sqrt)
                for qq in range(L // QT):
                    sl = slice(qq * QT, (qq + 1) * QT)
                    nc.vector.reciprocal(cbh[:, sl], t2s[qq][:])

                nc.vector.tensor_mul(rawh[:], rawh[:], cbh[:])
                nc.vector.scalar_tensor_tensor(
                    gt[:], rawh[:], nw[:], sgh[:],
                    op0=OP.mult, op1=OP.mult)
                nc.sync.dma_start(gstage[h][:], gt[:])

        # ================= Phase C: o_proj =================
        with ExitStack() as ctx:
            wop = ctx.enter_context(tc.tile_pool(name="wo", bufs=1))
            gpool = ctx.enter_context(tc.tile_pool(name="gres", bufs=1))
            wot, gres = [], []
            for h in range(HPC):
                t = wop.tile([P, HID], f32r, tag=f"wo{h}")
                nc.sync.dma_start(t[:], woT[h * P:(h + 1) * P, :])
                wot.append(t)
                g = gpool.tile([P, L], f32r, tag=f"gr{h}")
                nc.sync.dma_start(g[:], gstage[h][:])
                gres.append(g)
            opp = ctx.enter_context(
                tc.tile_pool(name="oproj_psum", bufs=2, space="PSUM"))
            oev = ctx.enter_context(tc.tile_pool(name="oev", bufs=3))
            for mc in range(L // P):
                ops = opp.tile([P, HID], f32, tag="op")
                for h in range(HPC):
                    for s in range(HID // QT):
                        nc.tensor.matmul(
                            ops[:, s * QT:(s + 1) * QT],
                            gres[h][:, mc * P:(mc + 1) * P],
                            wot[h][:, s * QT:(s + 1) * QT],
                            start=(h == 0),
                            stop=(h == HPC - 1),
                        )
                ot = oev.tile([P, HID], f32, tag="ot")
                nc.scalar.copy(ot[:], ops[:])
                nc.sync.dma_start(out_partial[mc * P:(mc + 1) * P, :], ot[:])

    return nc


def _rope_tables():
    inv_freq = 1.0 / (ROPE_BASE ** (np.arange(0, HD, 2, dtype=np.float64) / HD))
    t = np.arange(L, dtype=np.float64)
    f = np.outer(inv_freq, t)                      # [64, L]
    cosT = np.concatenate([np.cos(f), np.cos(f)], 0)
    ssinT = np.concatenate([-np.sin(f), np.sin(f)], 0)
    cosq = np.ascontiguousarray((cosT * SCALE).astype(np.float32))
    ssinq = np.ascontiguousarray((ssinT * SCALE).astype(np.float32))
    cosk = np.ascontiguousarray(cosT.astype(np.float32))
    ssink = np.ascontiguousarray(ssinT.astype(np.float32))
    return cosq, ssinq, cosk, ssink


def _static_in_maps(wq, wk, wv, wg, wo, norm_w):
    """Per-core input maps for everything except xT (static across calls)."""
    cosq, ssinq, cosk, ssink = _rope_tables()
    ones = np.ones((P, P), np.float32)
    oneshd = np.full((P, P), 1.0 / HD, np.float32)
    ident = np.eye(P, dtype=np.float32)
    qq = np.arange(QT)[None, :]
    kk = np.arange(P)[:, None]
    masks = np.ascontiguousarray(
        np.stack([(qq >= P * r + kk) for r in range(4)]).astype(np.float32))
    nw = np.ascontiguousarray(norm_w.astype(np.float32).reshape(P, 1))

    per_hg = []
    for hg in range(4):
        hs = slice(NDIM * hg, NDIM * (hg + 1))
        W = np.concatenate([wq[hs], wk[hs], wv[hs], wg[hs]], 0)
        wT = np.ascontiguousarray(np.asarray(W).T.astype(np.float32))
        wTb = np.ascontiguousarray(
            wT.reshape(KC, P, NCH, P).transpose(0, 2, 1, 3))
        woTc = np.ascontiguousarray(np.asarray(wo)[:, hs].T.astype(np.float32))
        per_hg.append((wTb, woTc))

    in_maps = []
    for c in range(NCORES):
        wTb, woTc = per_hg[c % 4]
        in_maps.append({
            "wTb": wTb, "woT": woTc,
            "cosq": cosq, "ssinq": ssinq, "cosk": cosk, "ssink": ssink,
            "ones_t": ones, "oneshd_t": oneshd, "ident_t": ident,
            "masks_t": masks, "nw_t": nw,
        })
    return in_maps


_ST = {}


def _get_runner(groups=(0, 1)):
    if "runner" in _ST:
        return _ST["runner"]

    import jax
    import jax.numpy as jnp
    from jax.sharding import Mesh, PartitionSpec as PS, NamedSharding
    try:
        from jax import shard_map as _sm

        def shard_map(f, **kw):
            return _sm(f, check_vma=False, **kw)
    except ImportError:
        from jax.experimental.shard_map import shard_map as _sm

        def shard_map(f, **kw):
            return _sm(f, check_rep=False, **kw)
    import concourse.bacc as bacc
    import concourse.mybir as mybir
    import concourse.tile as tile
    from concourse.bass2jax import (
        _bass_exec_p, install_neuronx_cc_hook, partition_id_tensor)

    install_neuronx_cc_hook()

    nc = bacc.Bacc("TRN2", target_bir_lowering=False, debug=False)
    _build(nc, mybir, tile)
    nc.compile()

    devs = jax.devices()[:NCORES]
    assert len(devs) == NCORES
    meshes = [Mesh(np.asarray(devs[4 * b:4 * b + 4]), ("h",))
              for b in range(B)]

    # Enumerate NEFF-bound tensors in BIR allocation order, exactly as
    # run_bass_via_pjrt does.
    partition_name = (nc.partition_id_tensor.name
                      if nc.partition_id_tensor else None)
    in_names, out_names, out_avals = [], [], []
    for alloc in nc.m.functions[0].allocations:
        if not isinstance(alloc, mybir.MemoryLocationSet):
            continue
        name = alloc.memorylocations[0].name
        if alloc.kind == "ExternalInput":
            if name != partition_name:
                in_names.append(name)
        elif alloc.kind == "ExternalOutput":
            out_avals.append(jax.core.ShapedArray(
                tuple(alloc.tensor_shape), mybir.dt.np(alloc.dtype)))
            out_names.append(name)
    n_params = len(in_names)
    in_shapes = {}
    for alloc in nc.m.functions[0].allocations:
        if isinstance(alloc, mybir.MemoryLocationSet) and alloc.kind == "ExternalInput":
            in_shapes[alloc.memorylocations[0].name] = tuple(alloc.tensor_shape)
    bind_in_names = list(in_names)
    if partition_name is not None:
        bind_in_names.append(partition_name)

    def _body(*args):
        operands = list(args)
        if partition_name is not None:
            operands.append(partition_id_tensor())
        outs = _bass_exec_p.bind(
            *operands,
            out_avals=tuple(out_avals),
            in_names=tuple(bind_in_names),
            out_names=tuple(out_names),
            lowering_input_output_aliases=(),
            sim_require_finite=True,
            sim_require_nnan=True,
            nc=nc,
        )
        return tuple(outs)

    def _spec(rank):
        return PS("h", *([None] * (rank - 1)))

    bass_in_specs = tuple(_spec(len(in_shapes[n])) for n in in_names)
    bass_out_specs = tuple(_spec(len(a.shape)) for a in out_avals)

    # prep: per-device [L/4, HID] fp16 slice of the batch -> full x[b]^T f32
    def _prep_body(xl):
        xg = jax.lax.all_gather(xl, "h", axis=0, tiled=True)   # [L, HID]
        return xg.astype(jnp.float32).T                         # [HID, L]

    # reduce: sum o_proj partials over the 4 head-group cores, scatter rows
    def _red_body(y):
        z = jax.lax.psum_scatter(y, "h", scatter_dimension=0, tiled=True)
        return z.astype(jnp.float16)

    bass_jits, prep_jits, red_jits, x_shs, g_shs = (
        [None] * B, [None] * B, [None] * B, [None] * B, [None] * B)
    for b in groups:
        mesh = meshes[b]
        bass_jits[b] = jax.jit(shard_map(
            _body, mesh=mesh, in_specs=bass_in_specs,
            out_specs=bass_out_specs))
        prep_jits[b] = jax.jit(shard_map(
            _prep_body, mesh=mesh, in_specs=(PS("h", None),),
            out_specs=PS("h", None)))
        red_jits[b] = jax.jit(shard_map(
            _red_body, mesh=mesh, in_specs=(PS("h", None),),
            out_specs=PS("h", None)))
        x_shs[b] = NamedSharding(mesh, PS("h", None))
        g_shs[b] = lambda rank, m=mesh: NamedSharding(m, _spec(rank))

    from concurrent.futures import ThreadPoolExecutor
    runner = {
        "jax": jax, "meshes": meshes, "x_sh": x_shs, "groups": tuple(groups),
        "in_names": in_names, "global_sharding": g_shs,
        "prep": prep_jits, "bass": bass_jits, "red": red_jits,
        "pool": ThreadPoolExecutor(max_workers=2 * NCORES),
    }
    _ST["runner"] = runner
    return runner


def _get_statics(runner, wq, wk, wv, wg, wo, norm_w):
    key = (id(wq), id(wk), id(wv), id(wg), id(wo), id(norm_w))
    cached = _ST.get("statics")
    if cached is not None and cached[0] == key:
        return cached[1]
    jax = runner["jax"]
    in_maps = _static_in_maps(np.asarray(wq), np.asarray(wk), np.asarray(wv),
                              np.asarray(wg), np.asarray(wo),
                              np.asarray(norm_w))
    statics = [None] * B
    for b in runner["groups"]:
        sb = {}
        for name in runner["in_names"]:
            if name == "xT":
                continue
            g = np.concatenate(
                [in_maps[c][name] for c in range(4 * b, 4 * b + 4)], axis=0)
            sb[name] = jax.device_put(g, runner["global_sharding"][b](g.ndim))
        statics[b] = sb
    jax.block_until_ready(
        [v for sb in statics if sb for v in sb.values()])
    _ST["statics"] = (key, statics)
    return statics


def _fetch_shard_into(sdata, dst_rows):
    dst_rows[...] = np.asarray(sdata).astype(np.float32)


def _run_batch_group(runner, statics, b, x_b, out_b, pool):
    """Upload x_b (fp16 [L, HID]), run prep/bass/red for group b, fetch
    the fp16 result into out_b (fp32 [L, HID]). Returns fetch futures."""
    jax = runner["jax"]
    xd = jax.device_put(x_b, runner["x_sh"][b])
    xT = runner["prep"][b](xd)
    args = [xT if n == "xT" else statics[b][n] for n in runner["in_names"]]
    (partials,) = runner["bass"][b](*args)
    o16 = runner["red"][b](partials)              # [L, HID] fp16, 4 shards
    o16.copy_to_host_async()
    return [pool.submit(_fetch_shard_into, s.data, out_b[s.index[0]])
            for s in o16.addressable_shards]


def _kernel_fast(hidden_states, wq, wk, wv, wg, wo, norm_w):
    runner = _get_runner()
    statics = _get_statics(runner, wq, wk, wv, wg, wo, norm_w)
    pool = runner["pool"]
    x = np.asarray(hidden_states)
    out = np.empty((B, L, HID), np.float32)
    futs = []
    for b in range(B):
        futs += _run_batch_group(runner, statics, b, x[b].astype(np.float16),
                                 out[b], pool)
    for f in futs:
        f.result()
    return out


# ---------------- fallback path (original runner) ----------------

def _host_inputs_full(hidden_states, wq, wk, wv, wg, wo, norm_w):
    x = np.ascontiguousarray(hidden_states.astype(np.float32))
    in_maps = _static_in_maps(wq, wk, wv, wg, wo, norm_w)
    for c in range(NCORES):
        in_maps[c] = dict(in_maps[c])
        in_maps[c]["xT"] = np.ascontiguousarray(x[c // 4].T)
    return in_maps


def _kernel_fallback(hidden_states, wq, wk, wv, wg, wo, norm_w):
    from concourse.bass_utils import run_bass_kernel_spmd
    import concourse.bacc as bacc
    import concourse.mybir as mybir
    import concourse.tile as tile

    if "nc_fb" not in _ST:
        nc = bacc.Bacc("TRN2", target_bir_lowering=False, debug=False)
        _build(nc, mybir, tile)
        nc.compile()
        _ST["nc_fb"] = nc
    nc = _ST["nc_fb"]
    in_maps = _host_inputs_full(np.asarray(hidden_states), np.asarray(wq),
                                np.asarray(wk), np.asarray(wv),
                                np.asarray(wg), np.asarray(wo),
                                np.asarray(norm_w))
    res = run_bass_kernel_spmd(nc, in_maps, list(range(NCORES)))
    out = np.zeros((B, L, HID), np.float32)
    for c in range(NCORES):
        out[c // 4] += res.results[c]["out_partial"]
    return out


def kernel(hidden_states, wq, wk, wv, wg, wo, norm_w, _trace=False):
    if not _ST.get("use_fallback"):
        try:
            return _kernel_fast(hidden_states, wq, wk, wv, wg, wo, norm_w)
        except Exception:
            import traceback
            traceback.print_exc()
            _ST["use_fallback"] = True
    return _kernel_fallback(hidden_states, wq, wk, wv, wg, wo, norm_w)
